# revision 2
# baseline (speedup 1.0000x reference)
"""AutoDiscretizationEmbedding kernel for 8 Trainium2 NeuronCores — v5.

Math per token t (x_t scalar):  h = leaky_relu(x_t*w1 + b1, 0.1);
logits = h + h@w2.T + b2;  out_t = softmax(logits) @ emb.

v5 = v4 (bf16 matmuls, z-first + batched reciprocal, po=6 PSUM bufs) plus
prologue compression:
  * x loads on the sync HWDGE ring; weights packed into two tensors loaded
    on the scalar HWDGE ring — separate DMA sem lanes, so the first
    broadcast only waits for the (small) x load.
  * ramp-up schedule: chunks of 128,128,128,128,256,256 tokens before the
    full 512-token chunks, so the first output store issues ~10us earlier
    and the store stream (the HBM-bound 46.4us) starts sooner.
"""

import numpy as np

B, S = 8, 8192
BINS, DIM = 100, 512
NCORES = 8
NTOK = (B * S) // NCORES
CHUNK = 512
NCH = NTOK // CHUNK

# ramp-up widths, full chunks, ramp-down (cheap final-store receipt);
# sums to NTOK
WIDTHS = ([128, 128, 128, 128, 256, 256] + [CHUNK] * (NCH - 3)
          + [256, 128, 128])
XHEAD = 1024  # first tokens loaded as a separate fast DMA

_CACHE = {}


def _build_nc():
    import concourse.tile as tile
    from concourse import bacc, mybir

    f32 = mybir.dt.float32
    bf16 = mybir.dt.bfloat16
    Act = mybir.ActivationFunctionType
    Alu = mybir.AluOpType

    nc = bacc.Bacc("TRN2", target_bir_lowering=False, debug=False,
                   num_devices=NCORES)
    xh_d = nc.dram_tensor("xh", [1, XHEAD], f32, kind="ExternalInput").ap()
    xt_d = nc.dram_tensor("xt", [1, NTOK - XHEAD], f32,
                          kind="ExternalInput").ap()
    smallw_d = nc.dram_tensor("smallw", [BINS, 3], f32,
                              kind="ExternalInput").ap()
    bigw_d = nc.dram_tensor("bigw", [BINS, 644], bf16,
                            kind="ExternalInput").ap()
    out_d = nc.dram_tensor("out", [NTOK, DIM], f32, kind="ExternalOutput").ap()

    with tile.TileContext(nc) as tc:
        with (
            tc.tile_pool(name="const", bufs=1) as cpool,
            tc.tile_pool(name="xb", bufs=3) as xbpool,
            tc.tile_pool(name="hsb", bufs=3) as hspool,
            tc.tile_pool(name="hT", bufs=3) as hpool,
            tc.tile_pool(name="uT", bufs=3) as upool,
            tc.tile_pool(name="rc", bufs=3) as rpool,
            tc.tile_pool(name="ost", bufs=4) as opool,
            tc.tile_pool(name="pl", bufs=1, space="PSUM") as pl,
            tc.tile_pool(name="pz", bufs=1, space="PSUM") as pz,
            tc.tile_pool(name="po", bufs=6, space="PSUM") as po,
        ):
            xo = cpool.tile([1, NTOK], f32)
            nc.sync.dma_start(xo[0:1, 0:XHEAD], xh_d[:])
            nc.sync.dma_start(xo[0:1, XHEAD:NTOK], xt_d[:])
            smallw = cpool.tile([BINS, 3], f32)
            nc.scalar.dma_start(smallw[:], smallw_d[:])
            bigw = cpool.tile([BINS, 644], bf16)
            nc.scalar.dma_start(bigw[:], bigw_d[:])
            w2ti = bigw[:, 0:128]
            embz = bigw[:, 128:644]
            w1c = smallw[:, 0:1]
            b1c = smallw[:, 1:2]
            b2c = smallw[:, 2:3]

            def stage_a(t0, w):
                xb = xbpool.tile([BINS, w], f32)
                nc.gpsimd.partition_broadcast(xb[:], xo[0:1, t0:t0 + w],
                                              channels=BINS)
                h_sb = hspool.tile([BINS, w], f32)
                nc.gpsimd.tensor_scalar(h_sb[:], xb[:], w1c, b1c,
                                        op0=Alu.mult, op1=Alu.add)
                hT = hpool.tile([BINS, w], bf16)
                nc.vector.scalar_tensor_tensor(hT[:], h_sb[:], 0.1, h_sb[:],
                                               op0=Alu.mult, op1=Alu.max)
                l_ps = pl.tile([128, w], f32)
                nc.tensor.matmul(l_ps[:], w2ti, hT[:], start=True, stop=True)
                uT = upool.tile([BINS, w], bf16)
                nc.scalar.activation(uT[:], l_ps[0:BINS, :], Act.Exp,
                                     bias=b2c)
                return uT

            def stage_b(t0, w, uT):
                nsub = w // 128
                # tiny z matmuls first so the reciprocal (and with it the
                # evictions) unblock before the o matmuls finish
                z_ps = pz.tile([128, nsub], f32)
                for j in range(nsub):
                    nc.tensor.matmul(z_ps[:, j:j + 1],
                                     uT[:, j * 128:(j + 1) * 128],
                                     embz[:, DIM + j:DIM + j + 1],
                                     start=True, stop=True)
                rc = rpool.tile([128, nsub], f32)
                nc.vector.reciprocal(rc[:], z_ps[:])

                o_list = []
                for j in range(nsub):
                    o_ps = po.tile([128, DIM], f32)
                    nc.tensor.matmul(o_ps[:], uT[:, j * 128:(j + 1) * 128],
                                     embz[:, 0:DIM], start=True, stop=True)
                    o_list.append(o_ps)

                ost = opool.tile([128, nsub * DIM], f32)
                for j in range(nsub):
                    dst = ost[:, j * DIM:(j + 1) * DIM]
                    if j % 2 == 0:
                        nc.scalar.activation(dst, o_list[j][:], Act.Copy,
                                             scale=rc[:, j:j + 1])
                    else:
                        nc.vector.tensor_scalar_mul(dst, o_list[j][:],
                                                    rc[:, j:j + 1])

                out_view = out_d[t0:t0 + w, :].rearrange(
                    "(a p) d -> p a d", p=128)
                nc.sync.dma_start(
                    out_view, ost[:].rearrange("p (a d) -> p a d", d=DIM))

            # software-pipelined emission: stage A of chunk i+1 gets a higher
            # scheduler priority than stage B of chunk i
            offs = [0]
            for w in WIDTHS:
                offs.append(offs[-1] + w)
            uts = [stage_a(offs[0], WIDTHS[0])]
            for i, w in enumerate(WIDTHS):
                if i + 1 < len(WIDTHS):
                    uts.append(stage_a(offs[i + 1], WIDTHS[i + 1]))
                stage_b(offs[i], w, uts[i])
    nc.compile()
    return nc


def _prep_in_maps(x, w1, b1, w2, b2, emb):
    x = np.ascontiguousarray(np.asarray(x, dtype=np.float32)).reshape(B * S)
    w1 = np.asarray(w1, dtype=np.float32)
    b1 = np.asarray(b1, dtype=np.float32)
    w2 = np.asarray(w2, dtype=np.float32)
    b2 = np.asarray(b2, dtype=np.float32)
    emb = np.asarray(emb, dtype=np.float32)

    import ml_dtypes
    bf16 = ml_dtypes.bfloat16

    smallw = np.ascontiguousarray(
        np.stack([w1[:, 0], b1, b2], axis=1))  # [BINS, 3] f32
    w2ti = np.zeros((BINS, 128), dtype=np.float32)
    w2ti[:, :BINS] = (w2 + np.eye(BINS, dtype=np.float32)).T
    embz = np.concatenate([emb, np.ones((BINS, 4), np.float32)], axis=1)
    bigw = np.ascontiguousarray(
        np.concatenate([w2ti, embz], axis=1)).astype(bf16)  # [BINS, 644]

    in_maps = []
    for c in range(NCORES):
        xc = x[c * NTOK:(c + 1) * NTOK]
        xh = np.ascontiguousarray(xc[:XHEAD][None, :])
        xt = np.ascontiguousarray(xc[XHEAD:][None, :])
        in_maps.append({"xh": xh, "xt": xt, "smallw": smallw, "bigw": bigw})
    return in_maps


def _run(in_maps, trace=False, **kw):
    from concourse.bass_utils import run_bass_kernel_spmd
    if "nc" not in _CACHE:
        _CACHE["nc"] = _build_nc()
    return run_bass_kernel_spmd(_CACHE["nc"], in_maps,
                                list(range(NCORES)), trace=trace, **kw)


def kernel(**inputs):
    in_maps = _prep_in_maps(inputs["x"], inputs["w1"], inputs["b1"],
                            inputs["w2"], inputs["b2"], inputs["emb"])
    res = _run(in_maps)
    out = np.stack([res.results[c]["out"] for c in range(NCORES)])
    return out.reshape(B, S, DIM).astype(np.float32, copy=False)
